# revision 1
# baseline (speedup 1.0000x reference)
"""Trainium2 Bass kernel for nn_Attention_29738353557815.

8-way tensor-parallel over heads:
  - core c owns q-heads {2c, 2c+1} and kv-head c//2 (k/v proj duplicated per core pair)
  - projections run weights-stationary off a host-pretransposed hidden^T, producing
    q/k in [head_dim, T] layout; v is produced transposed then PE-transposed back
  - rms-norm folded into ln/exp on ACT; rope tables (cos/sin * norm_w * sqrt(scale))
    are host-precomputed in [hd, T] layout; rotate-half via half-tile tensor_tensor
    ops against a half-swapped sin table
  - attention computed in S^T layout ([key, query] tiles): causal mask via
    gpsimd.affine_select, segment mask via scalar_tensor_tensor against iota;
    invalid (s,t) tiles are skipped entirely at build time (segment sparsity)
  - softmax denominator via ones-matmul column sums; normalization and sigmoid
    gating fused into one multiply before the o-projection
  - AllToAll (2 MiB/rank) redistributes gated attention so each core computes
    output rows [256c, 256c+256) with the full wo; host concatenates

All DMAs are arranged for >=4 KiB contiguous per-partition runs (weights are
host-prepacked into [128, ...] partition-major layouts) — smaller runs hit the
~200ns/descriptor DMA floor and halve effective bandwidth.
"""
import sys

if "/opt/trn_rl_repo" not in sys.path:
    sys.path.insert(0, "/opt/trn_rl_repo")

import numpy as np

import concourse.bass as bass
from concourse import bacc
import concourse.mybir as mybir
import concourse.tile as tile
from concourse.bass_utils import run_bass_kernel_spmd
from concourse.masks import make_identity

F32 = mybir.dt.float32
F32R = mybir.dt.float32r
BF16 = mybir.dt.float16  # fp16: same DMA savings as bf16, 4x finer mantissa
BF16_OPROJ = True  # o-projection pipeline (wo, A2A payload) in fp16
AF = mybir.ActivationFunctionType
OP = mybir.AluOpType

B, T, D = 1, 2048, 2048
NH, NKV, HD = 16, 4, 128
EPS = 1e-6
SCALE = HD ** -0.5
NCORES = 8
P = 128
NJ = T // 512      # 4 t-chunks of 512
NT = T // P        # 16 s-tiles of 128
DT = D // P        # 16 contraction tiles
TSL = T // NCORES  # 256 output rows per core

_program_cache: dict = {}


def _tile_flags(seg_end: np.ndarray):
    """Per (s-tile i, t-chunk j): (skip, needs_causal, needs_seg)."""
    flags = []
    for i in range(NT):
        smin, smax = P * i, P * i + P - 1
        se_lo = int(seg_end[smin])
        se_hi = int(seg_end[smax])
        row = []
        for j in range(NJ):
            t0, t1 = 512 * j, 512 * j + 511
            skip = (t1 < smin) or (t0 >= se_hi)
            causal = (not skip) and (t0 < smax)
            segm = (not skip) and (t1 >= se_lo)
            row.append((skip, causal, segm))
        flags.append(row)
    return tuple(tuple(r) for r in flags)


def _build_program(key, use_collective=True):
    flags, unit_w = key
    nc = bacc.Bacc("TRN2", target_bir_lowering=False, debug=False,
                   num_devices=NCORES)

    hT_d = nc.dram_tensor("hT", [D, T], F32R, kind="ExternalInput")
    # host-prepacked partition-major weights (see _host_prep)
    wqg_d = nc.dram_tensor("wqg", [P, DT, 512], F32R, kind="ExternalInput")
    wkv_d = nc.dram_tensor("wkv", [P, DT, 256], F32R, kind="ExternalInput")
    if BF16_OPROJ:
        wo_d = nc.dram_tensor("wo", [P, NT, 2048], BF16, kind="ExternalInput")
    else:
        wo_d = nc.dram_tensor("wo", [P, NT, 2, 1024], F32R, kind="ExternalInput")
    ODT = BF16 if BF16_OPROJ else F32R
    tblq_d = nc.dram_tensor("tblq", [2, P, T], F32, kind="ExternalInput")
    if not unit_w:
        wqk_d = nc.dram_tensor("wqk", [P, 2], F32, kind="ExternalInput")
    iota_d = nc.dram_tensor("iota", [P, 512], F32, kind="ExternalInput")
    segrel_d = nc.dram_tensor("segrel", [P, NT, NJ], F32, kind="ExternalInput")
    out_d = nc.dram_tensor("out", [TSL, D], F32, kind="ExternalOutput")

    hT_re = hT_d.rearrange("(dt p) t -> p dt t", p=P)

    hw_b, tmp_b, ptp_b, tmp2_b, atall_b = 16, 5, 5, 2, 8
    with tile.TileContext(nc) as tc:
        with (
            tc.tile_pool(name="consts", bufs=1) as consts,
            tc.tile_pool(name="perm", bufs=1) as perm,
            tc.tile_pool(name="hw", bufs=hw_b) as hw,
            tc.tile_pool(name="tmp", bufs=tmp_b) as tmp,
            tc.tile_pool(name="ptp", bufs=ptp_b) as ptp,
            tc.tile_pool(name="ps", bufs=1, space="PSUM") as psp,
            tc.tile_pool(name="dram", bufs=1, space="DRAM") as dram,
        ):
            # ---- constants; DMA emission of the big ones is interleaved with
            # the first hT tiles inside phase B so the first matmul starts early
            wqg_sb = [consts.tile([P, 4, 512], F32R, tag="wqg", bufs=4,
                                  name=f"wqg{g}") for g in range(4)]
            wkv_sb = [consts.tile([P, 8, 256], F32R, tag="wkv", bufs=2,
                                  name=f"wkv{g}") for g in range(2)]

            def wq_ap(dt, col0):
                return wqg_sb[dt // 4][:, dt % 4, col0:col0 + 128]

            def wkv_ap(dt, col0):
                return wkv_sb[dt // 8][:, dt % 8, col0:col0 + 128]

            tb = {}
            tb_srcs = []
            for nm, idx in (("cq", 0), ("sq", 1)):
                t_ = consts.tile([P, T], F32, tag=f"tb_{nm}", name=f"tb_{nm}")
                tb_srcs.append((t_, tblq_d, idx))
                tb[nm] = t_
            tb["ck"], tb["sk"] = tb["cq"], tb["sq"]
            if not unit_w:
                wqk_sb = consts.tile([P, 2], F32)
                nc.sync.dma_start(wqk_sb[:], wqk_d[:])
            iota_sb = consts.tile([P, 512], F32)
            segrel_sb = consts.tile([P, NT, NJ], F32)
            ones_f32 = consts.tile([P, P], F32)
            nc.vector.memset(ones_f32[:], 1.0)
            ones_sb = consts.tile([P, P], F32R)
            nc.vector.tensor_copy(ones_sb[:], ones_f32[:])
            ident_sb = consts.tile([P, P], F32)
            make_identity(nc, ident_sb[:])
            eps_sb = consts.tile([P, 1], F32)
            nc.vector.memset(eps_sb[:], EPS)

            # ---- persistent activations ----
            qTr = [perm.tile([P, T], F32R, tag=f"qTr{h}", name=f"qTr{h}")
                   for h in range(2)]
            kTr = perm.tile([P, T], F32R, tag="kTr")
            gT = [perm.tile([P, T], F32, tag=f"gT{h}", name=f"gT{h}")
                  for h in range(2)]
            v_sb = perm.tile([P, NT, P], F32R, tag="v_sb")

            # split A2A by head: h0's collective runs while h1 attention computes
            a2a_in = [dram.tile([NCORES * P, TSL], ODT, name=f"a2a_in{h}")
                      for h in range(2)]
            a2a_in8 = [a.rearrange("(s r) t -> s r t", r=P) for a in a2a_in]
            a2a_out = [dram.tile([NCORES * P, TSL], ODT, name=f"a2a_out{h}")
                       for h in range(2)]

            def emit_attention(h, j):
                tsl = slice(512 * j, 512 * j + 512)
                valid = [i for i in range(NT) if not flags[i][j][0]]
                last = len(valid) - 1
                ot_ps = psp.tile([P, 512], F32, tag="acc", bufs=4,
                                 name=f"ot_{h}_{j}")
                rs_ps = psp.tile([P, 512], F32, tag="acc", bufs=4,
                                 name=f"rs_{h}_{j}")
                for idx, i in enumerate(valid):
                    _, needs_c, needs_s = flags[i][j]
                    st_ps = psp.tile([P, 512], F32, tag="mm", bufs=3,
                                     name=f"st_{h}_{j}_{i}")
                    nc.tensor.matmul(st_ps[:], kTr[:, P * i:P * i + P],
                                     qTr[h][:, tsl], start=True, stop=True)
                    pt = ptp.tile([P, 512], F32R, tag="pt", name=f"pt_{h}_{j}_{i}")
                    nc.scalar.activation(pt[:], st_ps[:], AF.Exp)
                    if needs_c:
                        nc.gpsimd.affine_select(
                            out=pt[:], in_=pt[:], pattern=[[1, 512]],
                            compare_op=OP.is_ge, fill=0.0,
                            base=512 * j - P * i, channel_multiplier=-1)
                    if needs_s:
                        nc.vector.scalar_tensor_tensor(
                            out=pt[:], in0=iota_sb[:],
                            scalar=segrel_sb[:, i, j:j + 1], in1=pt[:],
                            op0=OP.is_lt, op1=OP.mult)
                    nc.tensor.matmul(ot_ps[:], v_sb[:, i, :], pt[:],
                                     start=(idx == 0), stop=(idx == last))
                    nc.tensor.matmul(rs_ps[:], ones_sb[:], pt[:],
                                     start=(idx == 0), stop=(idx == last))

                # sig(g)/rowsum = exp(-(ln(1+e^-g) + ln(rowsum)));
                # gT already holds ln(1+e^-g) from phase B
                sg = tmp.tile([P, 512], F32, tag="tmp", name=f"sg_{h}_{j}")
                nc.scalar.activation(sg[:], rs_ps[:], AF.Ln)
                nc.vector.tensor_tensor(sg[:], sg[:], gT[h][:, tsl], OP.add)
                nc.scalar.activation(sg[:], sg[:], AF.Exp, scale=-1.0)
                ot_sb = tmp.tile([P, 512], F32, tag="tmp", name=f"otsb_{h}_{j}")
                nc.vector.tensor_copy(ot_sb[:], ot_ps[:])
                atg = tmp.tile([P, 512], ODT, tag="tmp2", bufs=tmp2_b,
                               name=f"atg_{h}_{j}")
                nc.vector.tensor_tensor(atg[:], ot_sb[:], sg[:], OP.mult)
                # stage into a2a_in[h]: chunk j covers shards 2j and 2j+1
                for half in range(2):
                    nc.sync.dma_start(
                        a2a_in8[h][2 * j + half, :, :],
                        atg[:, 256 * half:256 * half + 256])


            # ================= phase B: projections =================
            # t-halves of 1024 so hT tiles have 4 KiB runs at tolerable SBUF cost
            for half in range(2):
                hTt = []
                for dt in range(DT):
                    if half == 0 and dt % 4 == 0:
                        g = dt // 4
                        nc.sync.dma_start(wqg_sb[g][:],
                                          wqg_d[:, 4 * g:4 * g + 4, :])
                    t_ = hw.tile([P, 1024], F32R, tag="hw", name=f"hT_{half}_{dt}")
                    nc.sync.dma_start(
                        t_[:], hT_re[:, dt, 1024 * half:1024 * half + 1024])
                    hTt.append(t_)
                if half == 0:
                    for g in range(2):
                        nc.sync.dma_start(wkv_sb[g][:],
                                          wkv_d[:, 8 * g:8 * g + 8, :])
                    for t_, dsrc, idx in tb_srcs:
                        nc.sync.dma_start(t_[:], dsrc[idx])
                    nc.sync.dma_start(iota_sb[:], iota_d[:])
                    nc.sync.dma_start(segrel_sb[:], segrel_d[:])
                for jj in range(2):
                    j = 2 * half + jj
                    tsl = slice(512 * j, 512 * j + 512)
                    hsl = slice(512 * jj, 512 * jj + 512)

                    # order: q0 q1 g0 g1 k v (k/v last -> slack for wkv DMA)
                    for c in (0, 1, 4, 5, 2, 3):
                        if c < 2:
                            w_ap = lambda dt, c=c: wq_ap(dt, 128 * c)
                        elif c == 2:
                            w_ap = lambda dt: wkv_ap(dt, 0)
                        elif c == 3:
                            w_ap = lambda dt: wkv_ap(dt, 128)
                        else:
                            w_ap = lambda dt, c=c: wq_ap(dt, 256 + 128 * (c - 4))

                        ptag, pbufs = (("mm", 3) if c in (0, 1, 4, 5) else ("acc", 4))
                        mm_ps = psp.tile([P, 512], F32, tag=ptag, bufs=pbufs,
                                         name=f"proj_{j}_{c}")
                        for dt in range(DT):
                            nc.tensor.matmul(mm_ps[:], w_ap(dt), hTt[dt][:, hsl],
                                             start=(dt == 0), stop=(dt == DT - 1))

                        if c in (0, 1, 2):  # q0/q1/k: rms-norm + rope
                            dest = qTr[c][:, tsl] if c < 2 else kTr[:, tsl]
                            cosw = tb["cq"] if c < 2 else tb["ck"]
                            sinw = tb["sq"] if c < 2 else tb["sk"]
                            qpre = tmp.tile([P, 512], F32, tag="tmp")
                            nc.vector.tensor_copy(qpre[:], mm_ps[:])
                            q2 = tmp.tile([P, 512], F32R, tag="tmp2", bufs=tmp2_b)
                            nc.scalar.activation(q2[:], mm_ps[:], AF.Square)
                            if not unit_w:
                                # norm weight applied after the rms statistic,
                                # before rope (rope commutes with rsqrt only)
                                qw = tmp.tile([P, 512], F32, tag="tmp")
                                nc.vector.tensor_scalar_mul(
                                    qw[:], qpre[:],
                                    wqk_sb[:, (0 if c < 2 else 1):
                                           (1 if c < 2 else 2)])
                                qpre = qw
                            ssq_ps = psp.tile([P, 512], F32, tag="aux", bufs=1)
                            nc.tensor.matmul(ssq_ps[:], ones_sb[:], q2[:],
                                             start=True, stop=True)
                            rsv = tmp.tile([P, 512], F32, tag="tmp")
                            nc.scalar.activation(rsv[:], ssq_ps[:], AF.Ln,
                                                 scale=1.0 / HD, bias=eps_sb[:, 0:1])
                            nc.scalar.activation(rsv[:], rsv[:], AF.Exp, scale=-0.5)
                            tcos = tmp.tile([P, 512], F32, tag="tmp")
                            nc.vector.tensor_tensor(tcos[:], qpre[:], cosw[:, tsl],
                                                    OP.mult)
                            t2 = tmp.tile([P, 512], F32, tag="tmp")
                            # sin table halves are pre-swapped host-side so both
                            # inputs share a base partition; only out is shifted
                            nc.vector.tensor_tensor(t2[0:64, :], qpre[64:128, :],
                                                    sinw[64:128, tsl], OP.mult)
                            nc.vector.tensor_tensor(t2[64:128, :], qpre[0:64, :],
                                                    sinw[0:64, tsl], OP.mult)
                            nc.vector.tensor_tensor(t2[:], tcos[:], t2[:], OP.add)
                            nc.vector.tensor_tensor(dest, t2[:], rsv[:], OP.mult)
                        elif c in (4, 5):  # gate: store ln(1+exp(-g))
                            eg = tmp.tile([P, 512], F32, tag="tmp")
                            nc.scalar.activation(eg[:], mm_ps[:], AF.Exp,
                                                 scale=-1.0)
                            nc.scalar.activation(gT[c - 4][:, tsl], eg[:],
                                                 AF.Ln, bias=1.0)
                        else:  # v: transpose [hd, t] -> [t, hd] tiles
                            vtmp = tmp.tile([P, 512], F32, tag="tmp")
                            nc.vector.tensor_copy(vtmp[:], mm_ps[:])
                            for kk in range(4):
                                tt = 4 * j + kk
                                trp = psp.tile([P, P], F32, tag="aux", bufs=1)
                                nc.tensor.transpose(
                                    trp[:], vtmp[:, 128 * kk:128 * kk + 128],
                                    ident_sb[:])
                                nc.vector.tensor_copy(v_sb[:, tt, :], trp[:])

            for j in range(NJ):
                emit_attention(0, j)
            if use_collective:
                nc.gpsimd.collective_compute(
                    "AllToAll", OP.bypass,
                    replica_groups=[list(range(NCORES))],
                    ins=[a2a_in[0][:].opt()], outs=[a2a_out[0][:].opt()])
            else:
                nc.sync.dma_start(a2a_out[0][:], a2a_in[0][:])

            # h=1 attention (h=0 was fused into the projection loop); its
            # collective overlaps with nothing ahead of it, while h=0's
            # collective ran during these blocks
            for j in range(NJ):
                emit_attention(1, j)
            if use_collective:
                nc.gpsimd.collective_compute(
                    "AllToAll", OP.bypass,
                    replica_groups=[list(range(NCORES))],
                    ins=[a2a_in[1][:].opt()], outs=[a2a_out[1][:].opt()])
            else:
                nc.sync.dma_start(a2a_out[1][:], a2a_in[1][:])

            # ================= phase D: o-proj =================

            # o-proj, ht-major: all 8 PSUM banks accumulate [m 0/1] x [Dc 0..3];
            # ATall and wo tiles stream (wo shares the "hw" slots freed by hT)
            ops_tags = ["mm", "mm", "mm", "aux", "acc", "acc", "acc", "acc"]
            ops_bufs = {"mm": 3, "aux": 1, "acc": 4}
            ops = []
            for m in range(2):
                for Dc in range(NJ):
                    tg = ops_tags[m * NJ + Dc]
                    ops.append(psp.tile([P, 512], F32, tag=tg,
                                        bufs=ops_bufs[tg], name=f"ops{m}_{Dc}"))
            # ht-step order: all h0 blocks then all h1 blocks (matches the
            # two collectives' completion order; wo is host-packed to match)
            ATall = []
            for hs in range(NT):
                h, i = hs // 8, hs % 8
                at_t = perm.tile([P, TSL], ODT, tag="ATall", bufs=atall_b,
                                 name=f"ATall{hs}")
                nc.sync.dma_start(at_t[:], a2a_out[h][P * i:P * i + P, :])
                ATall.append(at_t)
            for ht in range(NT):
                at_t = ATall[ht]
                if BF16_OPROJ:
                    w_full = hw.tile([P, 2048], BF16, tag="hw", name=f"wo_{ht}")
                    nc.sync.dma_start(w_full[:], wo_d[:, ht, :])
                    wslices = [w_full[:, 512 * Dc:512 * Dc + 512]
                               for Dc in range(NJ)]
                else:
                    wslices = []
                    for Dh in range(2):
                        w_ = hw.tile([P, 1024], F32R, tag="hw",
                                     name=f"wo_{ht}_{Dh}")
                        nc.sync.dma_start(w_[:], wo_d[:, ht, Dh, :])
                        wslices += [w_[:, 0:512], w_[:, 512:1024]]
                for Dc in range(NJ):
                    for m in range(2):
                        nc.tensor.matmul(
                            ops[m * NJ + Dc][:],
                            at_t[:, 128 * m:128 * m + 128], wslices[Dc],
                            start=(ht == 0), stop=(ht == NT - 1))
            # assemble [128, 1024] halves in freed "hw" slots so the final
            # writes have 4 KiB dram runs instead of floor-bound 2 KiB ones
            for m in range(2):
                for Dh in range(2):
                    o_sb = hw.tile([P, 1024], F32, tag="hw", name=f"o_{m}_{Dh}")
                    for q in range(2):
                        nc.vector.tensor_copy(o_sb[:, 512 * q:512 * q + 512],
                                              ops[m * NJ + 2 * Dh + q][:])
                    nc.sync.dma_start(
                        out_d[128 * m:128 * m + 128,
                              1024 * Dh:1024 * Dh + 1024], o_sb[:])

    nc.compile()
    _dedupe_act_table_loads(nc)
    return nc


def _dedupe_act_table_loads(nc):
    """Bacc assigns Exp->exp_and_others and Ln->natural_log, inserting a
    ~2.7us table load at every Exp<->Ln alternation (57 of them here). All
    activation funcs this kernel uses (Exp, Ln, Square) live in the
    natural_log_exp_and_others set, so keep one load of that set and drop
    the rest."""
    from concourse.hw_specs import get_activation_tables
    tabs = list(get_activation_tables(nc.m.arch).items())
    nl_exp = next(i for i, (nm, funcs) in enumerate(tabs)
                  if nm == "natural_log_exp_and_others")
    used = {ins.func for bb in nc.main_func.blocks for ins in bb.instructions
            if isinstance(ins, mybir.InstActivation)}
    assert used <= tabs[nl_exp][1], f"funcs {used} not all in natural_log_exp"
    first = True
    for bb in nc.main_func.blocks:
        keep = []
        for ins in bb.instructions:
            if isinstance(ins, mybir.InstLoadActFuncSet):
                assert ins.sync_info is None or (
                    not ins.sync_info.on_wait and not ins.sync_info.on_update)
                if first:
                    ins.act_func_set_id = nl_exp
                    keep.append(ins)
                    first = False
                continue
            keep.append(ins)
        bb.instructions[:] = keep


def _host_prep(hidden_BTD, cos_BTK, sin_BTK, segment_ids_BT, position_ids_BT,
               wq, wk, wv, wo, q_norm_w, k_norm_w):
    hidden = np.ascontiguousarray(np.asarray(hidden_BTD, dtype=np.float32)[0])
    cos = np.asarray(cos_BTK, dtype=np.float32)[0]
    sin = np.asarray(sin_BTK, dtype=np.float32)[0]
    seg = np.asarray(segment_ids_BT)[0]
    pos = np.asarray(position_ids_BT)[0]
    wq = np.asarray(wq, dtype=np.float32)
    wk = np.asarray(wk, dtype=np.float32)
    wv = np.asarray(wv, dtype=np.float32)
    wo = np.asarray(wo, dtype=np.float32)
    q_norm_w = np.asarray(q_norm_w, dtype=np.float32)
    k_norm_w = np.asarray(k_norm_w, dtype=np.float32)

    assert np.array_equal(pos, np.arange(T, dtype=pos.dtype)), \
        "kernel assumes position_ids == arange"
    assert np.all(np.diff(seg) >= 0), "kernel assumes sorted segment ids"

    hT = np.ascontiguousarray(hidden.T)
    sqrtS = np.float32(np.sqrt(SCALE))
    signv = np.where(np.arange(HD) < HD // 2, -1.0, 1.0).astype(np.float32)
    shuf = (np.arange(HD) + HD // 2) % HD

    cosw = (cos.T * sqrtS).astype(np.float32)
    sinw = (sin.T * signv[:, None] * sqrtS).astype(np.float32)
    sinswap = sinw[shuf]  # halves swapped: see rotate-half ops in _build_program
    tblq = np.ascontiguousarray(np.stack([cosw, sinswap]))
    unit_w = bool(np.all(q_norm_w == 1.0) and np.all(k_norm_w == 1.0))
    wqk = np.ascontiguousarray(np.stack([q_norm_w, k_norm_w], axis=1))

    # prepack wo into partition-major layout; block order matches the
    # o-proj ht-step order (all h0 head-blocks, then all h1)
    perm = [2 * i + h for h in range(2) for i in range(NCORES)]
    if BF16_OPROJ:
        wo_p = wo.reshape(NT, P, 2048)[perm].transpose(1, 0, 2)
        wo_p = np.ascontiguousarray(wo_p.astype(np.float16))
    else:
        wo_p = np.ascontiguousarray(
            wo.reshape(NT, P, 2, 1024)[perm].transpose(1, 0, 2, 3))

    seg_end = np.searchsorted(seg, seg, side="right").astype(np.int64)
    iota = np.broadcast_to(np.arange(512, dtype=np.float32), (P, 512)).copy()
    segrel = np.zeros((P, NT, NJ), dtype=np.float32)
    for i in range(NT):
        for j in range(NJ):
            segrel[:, i, j] = seg_end[P * i:P * i + P] - 512.0 * j

    in_maps = []
    for c in range(NCORES):
        h0, h1 = 2 * c, 2 * c + 1
        g = c // 2
        wqg = np.concatenate([
            wq[:, h0 * 256: h0 * 256 + 128],
            wq[:, h1 * 256: h1 * 256 + 128],
            wq[:, h0 * 256 + 128: h0 * 256 + 256],
            wq[:, h1 * 256 + 128: h1 * 256 + 256],
        ], axis=1)
        wqg_p = np.ascontiguousarray(wqg.reshape(DT, P, 512).transpose(1, 0, 2))
        wkv = np.concatenate([
            wk[:, g * 128:(g + 1) * 128], wv[:, g * 128:(g + 1) * 128]], axis=1)
        wkv_p = np.ascontiguousarray(wkv.reshape(DT, P, 256).transpose(1, 0, 2))
        m = {
            "hT": hT, "wqg": wqg_p, "wkv": wkv_p, "wo": wo_p,
            "tblq": tblq, "iota": iota, "segrel": segrel,
        }
        if not unit_w:
            m["wqk"] = wqk
        in_maps.append(m)
    return in_maps, seg_end, unit_w


def kernel(**inputs) -> np.ndarray:
    in_maps, seg_end, unit_w = _host_prep(**inputs)
    key = (_tile_flags(seg_end), unit_w)
    if key not in _program_cache:
        _program_cache[key] = _build_program(key)
    nc = _program_cache[key]
    res = run_bass_kernel_spmd(nc, in_maps, list(range(NCORES)))
    out = np.concatenate([res.results[c]["out"] for c in range(NCORES)], axis=0)
    return out[None].astype(np.float32)



# revision 6
# speedup vs baseline: 1.2512x; 1.2512x over previous
"""Trainium2 Bass kernel for nn_Attention_29738353557815.

8-way tensor-parallel over heads:
  - core c owns q-heads {2c, 2c+1} and kv-head c//2 (k/v proj duplicated per
    core pair); projections run weights-stationary off a host-pretransposed
    hidden^T in fp16, producing q/k in [head_dim, T] fp16 layout
  - rms-norm folded into ln/exp on ACT; rope tables (cos/sin * sqrt(scale))
    host-precomputed in [hd, T] layout; rotate-half via half-tile
    tensor_tensor ops against a half-swapped sin table
  - attention in S^T layout ([key, query] tiles), column-narrowed per tile to
    the valid [c0, c1) query range implied by causality and the (sorted)
    segment ids; exp has bias=-4 so fp16 probabilities cannot overflow
    (cancels between numerator and row-sum); softmax denominator via
    ones-matmul column sums; normalization and sigmoid gating fused into one
    multiply before the o-projection
  - phase order k/v/q0/g0 -> attn h0 -> AllToAll 0 -> q1/g1 -> attn h1 ->
    AllToAll 1 -> o-proj, so each 28us collective overlaps the other head's
    compute; o-proj consumes h0 blocks first so it starts right after coll0
  - o-proj writes psum accumulators straight to DRAM (2 KiB runs)

DMAs are >=512B-per-partition runs (below that the cost doubles); hT streams
as [128, 1024] fp16 tiles so the first projection group completes ~6us in.
"""
import sys

if "/opt/trn_rl_repo" not in sys.path:
    sys.path.insert(0, "/opt/trn_rl_repo")

import numpy as np

import concourse.bass as bass
from concourse import bacc
import concourse.mybir as mybir
import concourse.tile as tile
from concourse.bass_utils import run_bass_kernel_spmd
from concourse.masks import make_identity

F32 = mybir.dt.float32
F16 = mybir.dt.float16  # fp16: same speed/DMA as bf16, 4x finer mantissa
AF = mybir.ActivationFunctionType
OP = mybir.AluOpType

B, T, D = 1, 2048, 2048
NH, NKV, HD = 16, 4, 128
EPS = 1e-6
SCALE = HD ** -0.5
NCORES = 8
P = 128
NJ = T // 512      # 4 t-chunks of 512
NT = T // P        # 16 s-tiles of 128
DT = D // P        # 16 contraction tiles
TSL = T // NCORES  # 256 output rows per core
EXP_BIAS = -4.0    # exp(st-4): keeps fp16 probs < 65504; cancels in ratio

_program_cache: dict = {}


def _tile_flags(seg_end: np.ndarray):
    """Per (s-tile i, t-chunk j): None if skipped, else (c0, c1, needs_c,
    needs_s). Valid query cols are [c0, c1): c0 from causality (queries >=
    tile's first key), c1 from segments (all keys' segments end by
    seg_end(last key))."""
    out = []
    for i in range(NT):
        smin, smax = P * i, P * i + P - 1
        se_lo, se_hi = int(seg_end[smin]), int(seg_end[smax])
        row = []
        for j in range(NJ):
            c0 = max(0, P * i - 512 * j)
            c1 = min(512, se_hi - 512 * j)
            if c1 <= c0:
                row.append(None)
            else:
                needs_c = (P * i - 512 * j) >= 0      # diagonal tile
                needs_s = (se_lo - 512 * j) < c1      # seg boundary inside
                row.append((c0, c1, needs_c, needs_s))
        out.append(tuple(row))
    return tuple(out)


def _build_program(key, use_collective=True):
    flags, unit_w = key
    nc = bacc.Bacc("TRN2", target_bir_lowering=False, debug=False,
                   num_devices=NCORES)

    hT_d = nc.dram_tensor("hT", [D, T], F16, kind="ExternalInput")
    # host-prepacked partition-major weights (see _host_prep)
    wqg_d = nc.dram_tensor("wqg", [P, DT, 512], F16, kind="ExternalInput")
    wkv_d = nc.dram_tensor("wkv", [P, DT, 256], F16, kind="ExternalInput")
    wo_d = nc.dram_tensor("wo", [P, NT, 2048], F16, kind="ExternalInput")
    tblq_d = nc.dram_tensor("tblq", [2, P, T], F32, kind="ExternalInput")
    if not unit_w:
        wqk_d = nc.dram_tensor("wqk", [P, 2], F32, kind="ExternalInput")
    iota_d = nc.dram_tensor("iota", [P, 512], F16, kind="ExternalInput")
    segrel_d = nc.dram_tensor("segrel", [P, NT, NJ], F16, kind="ExternalInput")
    out_d = nc.dram_tensor("out", [TSL, D], F32, kind="ExternalOutput")

    hT_re = hT_d.rearrange("(dt p) t -> p dt t", p=P)

    with tile.TileContext(nc) as tc:
        with (
            tc.tile_pool(name="consts", bufs=1) as consts,
            tc.tile_pool(name="perm", bufs=1) as perm,
            tc.tile_pool(name="hw", bufs=32) as hw,
            tc.tile_pool(name="wop", bufs=8) as wop,
            tc.tile_pool(name="tmp", bufs=5) as tmp,
            tc.tile_pool(name="ptp", bufs=8) as ptp,
            tc.tile_pool(name="ps", bufs=1, space="PSUM") as psp,
            tc.tile_pool(name="dram", bufs=1, space="DRAM") as dram,
        ):
            # ---- constants ----
            wqg_sb = [consts.tile([P, 4, 512], F16, tag="wqg", bufs=4,
                                  name=f"wqg{g}") for g in range(4)]
            wkv_sb = [consts.tile([P, 8, 256], F16, tag="wkv", bufs=2,
                                  name=f"wkv{g}") for g in range(2)]

            def wq_ap(dt, col0):
                return wqg_sb[dt // 4][:, dt % 4, col0:col0 + 128]

            def wkv_ap(dt, col0):
                return wkv_sb[dt // 8][:, dt % 8, col0:col0 + 128]

            tb = {}
            for nm, idx in (("cq", 0), ("sq", 1)):
                tb[nm] = consts.tile([P, T], F32, tag=f"tb_{nm}", name=f"tb_{nm}")
            if not unit_w:
                wqk_sb = consts.tile([P, 2], F32)
            iota_sb = consts.tile([P, 512], F16)
            segrel_sb = consts.tile([P, NT, NJ], F16)
            ones_f32 = consts.tile([P, P], F32)
            ones_sb = consts.tile([P, P], F16)
            ident_sb = consts.tile([P, P], F16)
            eps_sb = consts.tile([P, 1], F32)
            ebias_sb = consts.tile([P, 1], F32)

            # ---- persistent activations ----
            qTr = [perm.tile([P, T], F16, tag=f"qTr{h}", name=f"qTr{h}")
                   for h in range(2)]
            kTr = perm.tile([P, T], F16, tag="kTr")
            gT = [perm.tile([P, T], F32, tag=f"gT{h}", name=f"gT{h}")
                  for h in range(2)]
            v_sb = perm.tile([P, NT, P], F16, tag="v_sb")

            # split A2A by head: h0's collective runs while h1 computes
            a2a_in = [dram.tile([NCORES * P, TSL], F16, name=f"a2a_in{h}")
                      for h in range(2)]
            a2a_in8 = [a.rearrange("(s r) t -> s r t", r=P) for a in a2a_in]
            a2a_out = [dram.tile([NCORES * P, TSL], F16, name=f"a2a_out{h}")
                       for h in range(2)]

            # ======== DMA emission (SP queue order = priority order) ========
            nc.sync.dma_start(wkv_sb[0][:], wkv_d[:, 0:8, :])
            hTt = [[None] * DT for _ in range(2)]
            for half in range(2):
                for dt in range(DT):
                    t_ = hw.tile([P, 1024], F16, tag="hw", bufs=32,
                                 name=f"hT_{half}_{dt}")
                    nc.sync.dma_start(
                        t_[:], hT_re[:, dt, 1024 * half:1024 * half + 1024])
                    hTt[half][dt] = t_
                    if half == 0:
                        if dt == 1:
                            for nm, idx in (("cq", 0), ("sq", 1)):
                                nc.sync.dma_start(tb[nm][:], tblq_d[idx])
                        if dt % 4 == 3:
                            g = dt // 4
                            nc.sync.dma_start(wqg_sb[g][:],
                                              wqg_d[:, 4 * g:4 * g + 4, :])
                        if dt == 8:
                            nc.sync.dma_start(wkv_sb[1][:], wkv_d[:, 8:16, :])
                        if dt == 12:
                            nc.sync.dma_start(iota_sb[:], iota_d[:])
                            nc.sync.dma_start(segrel_sb[:], segrel_d[:])
                            if not unit_w:
                                nc.sync.dma_start(wqk_sb[:], wqk_d[:])
            # o-proj weights, first 8 blocks prefetched (bufs=8)
            wo_sb = [None] * NT
            for ht in range(8):
                w_ = wop.tile([P, 2048], F16, tag="wop", bufs=8,
                              name=f"wo_{ht}")
                nc.sync.dma_start(w_[:], wo_d[:, ht, :])
                wo_sb[ht] = w_

            # ---- small on-chip constants ----
            nc.vector.memset(ones_f32[:], 1.0)
            nc.vector.tensor_copy(ones_sb[:], ones_f32[:])
            make_identity(nc, ident_sb[:])
            nc.vector.memset(eps_sb[:], EPS)
            nc.vector.memset(ebias_sb[:], EXP_BIAS)

            # ================= projections =================
            def emit_proj(c, j):
                """c: 0=q0 1=q1 2=k 3=v 4=g0 5=g1"""
                half, jj = j // 2, j % 2
                tsl = slice(512 * j, 512 * j + 512)
                hsl = slice(512 * jj, 512 * jj + 512)
                if c < 2:
                    w_ap = lambda dt: wq_ap(dt, 128 * c)
                elif c == 2:
                    w_ap = lambda dt: wkv_ap(dt, 0)
                elif c == 3:
                    w_ap = lambda dt: wkv_ap(dt, 128)
                else:
                    w_ap = lambda dt: wq_ap(dt, 256 + 128 * (c - 4))

                ptag, pbufs = (("mm", 3) if c in (0, 1, 4, 5) else ("acc", 4))
                mm_ps = psp.tile([P, 512], F32, tag=ptag, bufs=pbufs,
                                 name=f"proj_{j}_{c}")
                for dt in range(DT):
                    nc.tensor.matmul(mm_ps[:], w_ap(dt), hTt[half][dt][:, hsl],
                                     start=(dt == 0), stop=(dt == DT - 1))

                if c in (0, 1, 2):  # q0/q1/k: rms-norm + rope
                    dest = qTr[c][:, tsl] if c < 2 else kTr[:, tsl]
                    qpre = tmp.tile([P, 512], F32, tag="tmp")
                    nc.vector.tensor_copy(qpre[:], mm_ps[:])
                    q2 = tmp.tile([P, 512], F16, tag="tmp2", bufs=2)
                    nc.scalar.activation(q2[:], mm_ps[:], AF.Square)
                    if not unit_w:
                        # norm weight applied after the rms statistic,
                        # before rope (rope commutes with rsqrt only)
                        qw = tmp.tile([P, 512], F32, tag="tmp")
                        nc.vector.tensor_scalar_mul(
                            qw[:], qpre[:],
                            wqk_sb[:, (0 if c < 2 else 1):
                                   (1 if c < 2 else 2)])
                        qpre = qw
                    ssq_ps = psp.tile([P, 512], F32, tag="aux", bufs=1)
                    nc.tensor.matmul(ssq_ps[:], ones_sb[:], q2[:],
                                     start=True, stop=True)
                    rsv = tmp.tile([P, 512], F32, tag="tmp")
                    nc.scalar.activation(rsv[:], ssq_ps[:], AF.Ln,
                                         scale=1.0 / HD, bias=eps_sb[:, 0:1])
                    nc.scalar.activation(rsv[:], rsv[:], AF.Exp, scale=-0.5)
                    tcos = tmp.tile([P, 512], F32, tag="tmp")
                    nc.vector.tensor_tensor(tcos[:], qpre[:], tb["cq"][:, tsl],
                                            OP.mult)
                    t2 = tmp.tile([P, 512], F32, tag="tmp")
                    # sin table halves are pre-swapped host-side so both
                    # inputs share a base partition; only out is shifted
                    nc.vector.tensor_tensor(t2[0:64, :], qpre[64:128, :],
                                            tb["sq"][64:128, tsl], OP.mult)
                    nc.vector.tensor_tensor(t2[64:128, :], qpre[0:64, :],
                                            tb["sq"][0:64, tsl], OP.mult)
                    nc.vector.tensor_tensor(t2[:], tcos[:], t2[:], OP.add)
                    nc.vector.tensor_tensor(dest, t2[:], rsv[:], OP.mult)
                elif c in (4, 5):  # gate: store ln(1+exp(-g))
                    eg = tmp.tile([P, 512], F32, tag="tmp")
                    nc.scalar.activation(eg[:], mm_ps[:], AF.Exp, scale=-1.0)
                    nc.scalar.activation(gT[c - 4][:, tsl], eg[:],
                                         AF.Ln, bias=1.0)
                else:  # v: transpose [hd, t] -> [t, hd] tiles
                    vtmp = tmp.tile([P, 512], F16, tag="tmp2", bufs=2)
                    nc.vector.tensor_copy(vtmp[:], mm_ps[:])
                    for kk in range(4):
                        tt = 4 * j + kk
                        trp = psp.tile([P, P], F16, tag="aux", bufs=1)
                        nc.tensor.transpose(
                            trp[:], vtmp[:, 128 * kk:128 * kk + 128],
                            ident_sb[:])
                        nc.vector.tensor_copy(v_sb[:, tt, :], trp[:])

            # ================= attention =================
            def emit_attention(h, j):
                tsl0 = 512 * j
                valid = [(i,) + flags[i][j] for i in range(NT)
                         if flags[i][j] is not None]
                last = len(valid) - 1
                ot_ps = psp.tile([P, 512], F32, tag="acc", bufs=4,
                                 name=f"ot_{h}_{j}")
                rs_ps = psp.tile([P, 512], F32, tag="acc", bufs=4,
                                 name=f"rs_{h}_{j}")
                maxc1 = 0
                for idx, (i, c0, c1, needs_c, needs_s) in enumerate(valid):
                    st_ps = psp.tile([P, 512], F32, tag="mm", bufs=3,
                                     name=f"st_{h}_{j}_{i}")
                    nc.tensor.matmul(st_ps[:, c0:c1], kTr[:, P * i:P * i + P],
                                     qTr[h][:, tsl0 + c0:tsl0 + c1],
                                     start=True, stop=True)
                    pt = ptp.tile([P, 512], F16, tag="pt",
                                  name=f"pt_{h}_{j}_{i}")
                    nc.scalar.activation(pt[:, c0:c1], st_ps[:, c0:c1],
                                         AF.Exp, bias=ebias_sb[:, 0:1])
                    if needs_c:
                        nc.gpsimd.affine_select(
                            out=pt[:, c0:c1], in_=pt[:, c0:c1],
                            pattern=[[1, c1 - c0]],
                            compare_op=OP.is_ge, fill=0.0,
                            base=512 * j + c0 - P * i, channel_multiplier=-1)
                    if needs_s:
                        nc.vector.scalar_tensor_tensor(
                            out=pt[:, c0:c1], in0=iota_sb[:, c0:c1],
                            scalar=segrel_sb[:, i, j:j + 1], in1=pt[:, c0:c1],
                            op0=OP.is_lt, op1=OP.mult)
                    # coverage-split: first writer of each column range gets
                    # start=True (c0 is nondecreasing over valid tiles and
                    # every column's own diagonal tile is always valid, so
                    # ranges never leave gaps)
                    if c1 <= maxc1:
                        segs = [(c0, c1, False)]
                    elif c0 < maxc1:
                        segs = [(c0, maxc1, False), (maxc1, c1, True)]
                    else:
                        segs = [(c0, c1, True)]
                    for (a, b, st_flag) in segs:
                        nc.tensor.matmul(ot_ps[:, a:b], v_sb[:, i, :],
                                         pt[:, a:b], start=st_flag,
                                         stop=(idx == last),
                                         skip_group_check=True)
                        nc.tensor.matmul(rs_ps[:, a:b], ones_sb[:],
                                         pt[:, a:b], start=st_flag,
                                         stop=(idx == last),
                                         skip_group_check=True)
                    maxc1 = max(maxc1, c1)

                # sig(g)/rowsum = exp(-(ln(1+e^-g) + ln(rowsum)));
                # gT already holds ln(1+e^-g)
                tsl = slice(512 * j, 512 * j + 512)
                sg = tmp.tile([P, 512], F32, tag="tmp", name=f"sg_{h}_{j}")
                nc.scalar.activation(sg[:], rs_ps[:], AF.Ln)
                nc.vector.tensor_tensor(sg[:], sg[:], gT[h][:, tsl], OP.add)
                nc.scalar.activation(sg[:], sg[:], AF.Exp, scale=-1.0)
                ot_sb = tmp.tile([P, 512], F32, tag="tmp", name=f"otsb_{h}_{j}")
                nc.vector.tensor_copy(ot_sb[:], ot_ps[:])
                atg = tmp.tile([P, 512], F16, tag="tmp2", bufs=2,
                               name=f"atg_{h}_{j}")
                nc.vector.tensor_tensor(atg[:], ot_sb[:], sg[:], OP.mult)
                # stage into a2a_in[h]: chunk j covers shards 2j and 2j+1
                for half in range(2):
                    nc.sync.dma_start(
                        a2a_in8[h][2 * j + half, :, :],
                        atg[:, 256 * half:256 * half + 256])

            # ======== phase A: k/v/q0/g0 for all T, then h0 attention ======
            for j in range(NJ):
                for c in (2, 3, 0, 4):  # k, v, q0, g0
                    emit_proj(c, j)
            for j in range(NJ):
                emit_attention(0, j)
            if use_collective:
                nc.gpsimd.collective_compute(
                    "AllToAll", OP.bypass,
                    replica_groups=[list(range(NCORES))],
                    ins=[a2a_in[0][:].opt()], outs=[a2a_out[0][:].opt()])
            else:
                nc.sync.dma_start(a2a_out[0][:], a2a_in[0][:])

            # ======== phase C: q1/g1, then h1 attention (over coll0) =======
            for j in range(NJ):
                for c in (1, 5):  # q1, g1
                    emit_proj(c, j)
            for j in range(NJ):
                emit_attention(1, j)
            if use_collective:
                nc.gpsimd.collective_compute(
                    "AllToAll", OP.bypass,
                    replica_groups=[list(range(NCORES))],
                    ins=[a2a_in[1][:].opt()], outs=[a2a_out[1][:].opt()])
            else:
                nc.sync.dma_start(a2a_out[1][:], a2a_in[1][:])

            # ================= o-proj =================
            # ht order: all 8 h0 blocks (ready at coll0), then 8 h1 blocks;
            # wo is host-packed to match. ATall DMAs interleave with the
            # remaining wo loads so nothing dep-blocks the queue head.
            ATall = [None] * NT
            for hs in range(8):
                at_t = perm.tile([P, TSL], F16, tag="ATall", bufs=8,
                                 name=f"ATall{hs}")
                nc.sync.dma_start(at_t[:], a2a_out[0][P * hs:P * hs + P, :])
                ATall[hs] = at_t
            for ht in range(8, 12):
                w_ = wop.tile([P, 2048], F16, tag="wop", bufs=8,
                              name=f"wo_{ht}")
                nc.sync.dma_start(w_[:], wo_d[:, ht, :])
                wo_sb[ht] = w_
            for hs in range(8):
                at_t = perm.tile([P, TSL], F16, tag="ATall", bufs=8,
                                 name=f"ATall{8 + hs}")
                nc.sync.dma_start(at_t[:], a2a_out[1][P * hs:P * hs + P, :])
                ATall[8 + hs] = at_t
            for ht in range(12, NT):
                w_ = wop.tile([P, 2048], F16, tag="wop", bufs=8,
                              name=f"wo_{ht}")
                nc.sync.dma_start(w_[:], wo_d[:, ht, :])
                wo_sb[ht] = w_

            # all 8 PSUM banks accumulate [m 0/1] x [Dc 0..3]
            ops_tags = ["mm", "mm", "mm", "aux", "acc", "acc", "acc", "acc"]
            ops_bufs = {"mm": 3, "aux": 1, "acc": 4}
            ops = []
            for m in range(2):
                for Dc in range(NJ):
                    tg = ops_tags[m * NJ + Dc]
                    ops.append(psp.tile([P, 512], F32, tag=tg,
                                        bufs=ops_bufs[tg], name=f"ops{m}_{Dc}"))
            for ht in range(NT):
                at_t = ATall[ht]
                w_full = wo_sb[ht]
                for Dc in range(NJ):
                    for m in range(2):
                        nc.tensor.matmul(
                            ops[m * NJ + Dc][:],
                            at_t[:, 128 * m:128 * m + 128],
                            w_full[:, 512 * Dc:512 * Dc + 512],
                            start=(ht == 0), stop=(ht == NT - 1))
                        if ht == NT - 1:
                            # stream each finished accumulator out via SBUF
                            o_sb = tmp.tile([P, 512], F32, tag="osb", bufs=3,
                                            name=f"osb_{m}_{Dc}")
                            nc.vector.tensor_copy(o_sb[:], ops[m * NJ + Dc][:])
                            nc.sync.dma_start(
                                out_d[128 * m:128 * m + 128,
                                      512 * Dc:512 * Dc + 512],
                                o_sb[:])

    nc.compile()
    _dedupe_act_table_loads(nc)
    return nc


def _dedupe_act_table_loads(nc):
    """Bacc assigns Exp->exp_and_others and Ln->natural_log, inserting a
    ~2.7us table load at every Exp<->Ln alternation. All activation funcs
    this kernel uses (Exp, Ln, Square) live in the natural_log_exp_and_others
    set, so keep one load of that set and drop the rest."""
    from concourse.hw_specs import get_activation_tables
    tabs = list(get_activation_tables(nc.m.arch).items())
    nl_exp = next(i for i, (nm, funcs) in enumerate(tabs)
                  if nm == "natural_log_exp_and_others")
    used = {ins.func for bb in nc.main_func.blocks for ins in bb.instructions
            if isinstance(ins, mybir.InstActivation)}
    assert used <= tabs[nl_exp][1], f"funcs {used} not all in natural_log_exp"
    first = True
    for bb in nc.main_func.blocks:
        keep = []
        for ins in bb.instructions:
            if isinstance(ins, mybir.InstLoadActFuncSet):
                assert ins.sync_info is None or (
                    not ins.sync_info.on_wait and not ins.sync_info.on_update)
                if first:
                    ins.act_func_set_id = nl_exp
                    keep.append(ins)
                    first = False
                continue
            keep.append(ins)
        bb.instructions[:] = keep


def _host_prep(hidden_BTD, cos_BTK, sin_BTK, segment_ids_BT, position_ids_BT,
               wq, wk, wv, wo, q_norm_w, k_norm_w):
    hidden = np.ascontiguousarray(np.asarray(hidden_BTD, dtype=np.float32)[0])
    cos = np.asarray(cos_BTK, dtype=np.float32)[0]
    sin = np.asarray(sin_BTK, dtype=np.float32)[0]
    seg = np.asarray(segment_ids_BT)[0]
    pos = np.asarray(position_ids_BT)[0]
    wq = np.asarray(wq, dtype=np.float32)
    wk = np.asarray(wk, dtype=np.float32)
    wv = np.asarray(wv, dtype=np.float32)
    wo = np.asarray(wo, dtype=np.float32)
    q_norm_w = np.asarray(q_norm_w, dtype=np.float32)
    k_norm_w = np.asarray(k_norm_w, dtype=np.float32)

    assert np.array_equal(pos, np.arange(T, dtype=pos.dtype)), \
        "kernel assumes position_ids == arange"
    assert np.all(np.diff(seg) >= 0), "kernel assumes sorted segment ids"

    hT = np.ascontiguousarray(hidden.T.astype(np.float16))
    sqrtS = np.float32(np.sqrt(SCALE))
    signv = np.where(np.arange(HD) < HD // 2, -1.0, 1.0).astype(np.float32)
    shuf = (np.arange(HD) + HD // 2) % HD

    cosw = (cos.T * sqrtS).astype(np.float32)
    sinw = (sin.T * signv[:, None] * sqrtS).astype(np.float32)
    sinswap = sinw[shuf]  # halves swapped: see rotate-half ops in _build_program
    tblq = np.ascontiguousarray(np.stack([cosw, sinswap]))
    unit_w = bool(np.all(q_norm_w == 1.0) and np.all(k_norm_w == 1.0))
    wqk = np.ascontiguousarray(np.stack([q_norm_w, k_norm_w], axis=1))

    # prepack wo into partition-major layout; block order matches the
    # o-proj ht-step order (all h0 head-blocks, then all h1)
    perm = [2 * i + h for h in range(2) for i in range(NCORES)]
    wo_p = wo.reshape(NT, P, 2048)[perm].transpose(1, 0, 2)
    wo_p = np.ascontiguousarray(wo_p.astype(np.float16))

    seg_end = np.searchsorted(seg, seg, side="right").astype(np.int64)
    iota = np.broadcast_to(np.arange(512, dtype=np.float16), (P, 512)).copy()
    segrel = np.zeros((P, NT, NJ), dtype=np.float16)
    for i in range(NT):
        for j in range(NJ):
            segrel[:, i, j] = seg_end[P * i:P * i + P] - 512.0 * j

    in_maps = []
    for c in range(NCORES):
        h0, h1 = 2 * c, 2 * c + 1
        g = c // 2
        wqg = np.concatenate([
            wq[:, h0 * 256: h0 * 256 + 128],
            wq[:, h1 * 256: h1 * 256 + 128],
            wq[:, h0 * 256 + 128: h0 * 256 + 256],
            wq[:, h1 * 256 + 128: h1 * 256 + 256],
        ], axis=1).astype(np.float16)
        wqg_p = np.ascontiguousarray(wqg.reshape(DT, P, 512).transpose(1, 0, 2))
        wkv = np.concatenate([
            wk[:, g * 128:(g + 1) * 128], wv[:, g * 128:(g + 1) * 128]],
            axis=1).astype(np.float16)
        wkv_p = np.ascontiguousarray(wkv.reshape(DT, P, 256).transpose(1, 0, 2))
        m = {
            "hT": hT, "wqg": wqg_p, "wkv": wkv_p, "wo": wo_p,
            "tblq": tblq, "iota": iota, "segrel": segrel,
        }
        if not unit_w:
            m["wqk"] = wqk
        in_maps.append(m)
    return in_maps, seg_end, unit_w


def kernel(**inputs) -> np.ndarray:
    in_maps, seg_end, unit_w = _host_prep(**inputs)
    key = (_tile_flags(seg_end), unit_w)
    if key not in _program_cache:
        _program_cache[key] = _build_program(key)
    nc = _program_cache[key]
    res = run_bass_kernel_spmd(nc, in_maps, list(range(NCORES)))
    out = np.concatenate([res.results[c]["out"] for c in range(NCORES)], axis=0)
    return out[None].astype(np.float32)
